# revision 15
# baseline (speedup 1.0000x reference)
"""Distributed Trainium2 kernel for nn_Attention (B=2, N=2048, D=1024, H=16).

Sharding: tensor-parallel over heads (2 heads per core) for qkv + attention,
then an AllToAll redistributes attention output so each core projects a
512-row slice of the output (cores 0-3: batch 0, cores 4-7: batch 1).

v2 restructure (from trace analysis of the 282us baseline):
  - The attention phase is ACT(exp)-bound (~1126ns per [128,1024] block,
    back-to-back); the PE has ~40% slack there but the baseline ran qkv and
    proj as separate serial phases. Now only batch-0's minimal qkv prefix
    (Q0,K0,V0,K1,V1) runs as a prologue; every remaining qkv tile-group is
    fed into the attention iterations' PE slack via a (iter,kb) schedule
    that respects just-in-time deadlines (K/V blocks consumed at kb pace).
  - HAM kept warm: the PE's ~50% duty cycle let the activity monitor
    re-throttle it to 1.2GHz for 35..204us of the baseline (all attention
    matmuls ran 433ns instead of 216ns). Interleaved qkv raises duty in
    iters 0-4; a paced dummy-matmul chain bridges the AllToAll so proj
    starts at 2.4GHz instead of half clock.
  - AllToAll warm-up collective is pinned late (its input DMA sources
    iteration-6 data) instead of being hoisted to t=72us by the scheduler.
  - Both heads' softmax denominators land on psum partitions 0/32 (head-B
    ones column moved), so one DVE reciprocal [33,512] covers both heads
    (iterative divide is per-free-element: halves 6.6us/iter -> 3.3).
  - Per-core dataflow otherwise as baseline: Q^T/K^T [128, 4096] bf16,
    V packed in vones with ones-columns so PV also produces denominators;
    scores per 128k x 512q with two heads row-tiled concurrently on the PE;
    exp on ScalarE (no max subtraction; |s|<~7); deferred normalization.
"""

import sys
import types

import numpy as np

if "/opt/trn_rl_repo" not in sys.path:
    sys.path.insert(0, "/opt/trn_rl_repo")

import ml_dtypes

B, N, D = 2, 2048, 1024
H, HD = 16, 64
SCALE = HD**-0.5
TOK = B * N  # 4096, token index = b*N + t
EC = 8  # embed-dim chunks of 128
NCORES = 8
# per k-block vones layout [128 tok, 256]:
#   [1 | 0*63 | V_A(64) | 0*32 | 1 | 0*31 | V_B(64)]
# so the PV matmul (M=128) puts head-A's softmax denominator on psum
# partition 0 and head-B's on partition 32 (32-aligned so the DVE den
# copies stay partition-aligned and one [33,512] reciprocal covers both).
VSTRIDE = 256
NKB = TOK // 128  # 32 k-blocks across both batches
NTCN = TOK // 512  # 8 qkv token tiles

BF16 = ml_dtypes.bfloat16


def _install_axon_profile_hook():
    """Best-effort: register the NTFF profile hook the RL container's antenv
    stub omits, so run_bass_kernel_spmd(trace=True) can report exec_time_ns."""
    try:
        import antenv

        if "antenv.axon_hooks" not in sys.modules:
            hooks = types.ModuleType("antenv.axon_hooks")
            hooks._hook = None
            hooks.set_axon_ntff_profile_hook = lambda h: setattr(hooks, "_hook", h)
            hooks.get_axon_ntff_profile_hook = lambda: hooks._hook
            sys.modules["antenv.axon_hooks"] = hooks
            antenv.axon_hooks = hooks
            from trn_agent_boot.trn_boot import _ntff_profile_via_ctypes

            hooks.set_axon_ntff_profile_hook(
                _ntff_profile_via_ctypes("/opt/axon/libaxon_pjrt.so")
            )
        return True
    except Exception:
        return False


def _split_multi_waits(nc):
    """neuronxcc's walrus (CoreV3 setupSyncWait) rejects instructions that
    carry more than one semaphore wait, but Tile's wait assignment freely
    attaches several. Hoist the extra waits onto freshly inserted same-engine
    NoOps placed directly before the instruction — the engine stalls at the
    same program point, so semantics are unchanged."""
    import concourse.mybir as mybir

    n_split = 0
    for fn in nc.m.functions:
        for bb in fn.blocks:
            insts = bb.instructions
            if not any(
                i.sync_info is not None and len(i.sync_info.on_wait) > 1
                for i in insts
            ):
                continue
            new_insts = []
            for ins in insts:
                si = ins.sync_info
                if si is not None and len(si.on_wait) > 1:
                    waits = list(si.on_wait)
                    for w in waits[:-1]:
                        nop = mybir.InstNoOp(
                            name=f"wsplit-{n_split}",
                            engine=ins.engine,
                            ins=[],
                            outs=[],
                            sync_info=mybir.SyncInfo(on_wait=[w], on_update=[]),
                        )
                        new_insts.append(nop)
                        n_split += 1
                    ins.sync_info = mybir.SyncInfo(
                        on_wait=[waits[-1]], on_update=list(si.on_update)
                    )
                new_insts.append(ins)
            bb.instructions = new_insts


def _build_nc():
    import concourse.bass as bass
    import concourse.mybir as mybir
    import concourse.tile as tile

    F32 = mybir.dt.float32
    BF = mybir.dt.bfloat16
    AF = mybir.ActivationFunctionType
    ALU = mybir.AluOpType

    nc = bass.Bass()
    xT_ext = nc.declare_dram_parameter("xT", [D, TOK], BF, isOutput=False)
    wq_ext = nc.declare_dram_parameter("wq", [128, 1024], BF, isOutput=False)
    wk_ext = nc.declare_dram_parameter("wk", [128, 1024], BF, isOutput=False)
    wv_ext = nc.declare_dram_parameter("wv", [128, 1024], BF, isOutput=False)
    wp_ext = nc.declare_dram_parameter("wp", [128, 8192], BF, isOutput=False)
    bias_ext = nc.declare_dram_parameter("bias", [128, 8], F32, isOutput=False)
    coreid_ext = nc.declare_dram_parameter(
        "coreid", [1, 1], mybir.dt.uint32, isOutput=False
    )
    out_ext = nc.declare_dram_parameter("out", [D, 512], F32, isOutput=True)

    with tile.TileContext(nc) as tc:
        with (
            tc.tile_pool(name="const", bufs=1) as cpool,
            tc.tile_pool(name="e", bufs=6) as epool,
            tc.tile_pool(name="norm", bufs=2) as npool,
            tc.tile_pool(name="y", bufs=2) as ypool,
            tc.tile_pool(name="psum", bufs=2, space="PSUM") as psum,
            tc.tile_pool(name="dram", bufs=1, space="DRAM") as dram,
        ):
            wq_sb = cpool.tile([128, 1024], BF)
            wk_sb = cpool.tile([128, 1024], BF)
            wv_sb = cpool.tile([128, 1024], BF)
            wp_sb = cpool.tile([128, 8192], BF)
            bias_sb = cpool.tile([128, 8], F32)
            qt_sb = cpool.tile([128, TOK], BF)
            kt_sb = cpool.tile([128, TOK], BF)
            vones = cpool.tile([128, NKB, VSTRIDE], BF)
            # all 8 x token-tiles stay resident (qkv is spread over attention)
            x_sb = [
                cpool.tile([128, EC, 512], BF, name=f"x{t}") for t in range(NTCN)
            ]
            # garbage tiles for HAM keep-warm dummy matmuls (never written;
            # NaN results land in recycled psum slots and are never read)
            garb = cpool.tile([128, 512], BF)
            gout = cpool.tile([128, 512], BF)

            nc.vector.memset(garb[:], 0.0)
            nc.vector.memset(vones[:], 0.0)
            nc.vector.memset(vones[:, :, 0:1], 1.0)
            nc.vector.memset(vones[:, :, 160:161], 1.0)
            bones = cpool.tile([33, 64], BF)
            nc.vector.memset(bones[0:1, :], 1.0)
            nc.vector.memset(bones[32:33, :], 1.0)

            # ---------------- input DMA issue schedule ----------------
            # sync gets the weights + even x(b0) chunks; scalar (idle until
            # the first exp) gets the odd chunks. x(b1) is issued from
            # inside attention iterations 0-1 on sync; wp/bias mid-attention
            # (the prologue window is HBM-bandwidth limited: 8 cores pull
            # replicated x concurrently).
            for ec in range(0, EC, 2):
                nc.sync.dma_start(
                    x_sb[0][:, ec, :], xT_ext[ec * 128 : (ec + 1) * 128, 0:512]
                )
            nc.sync.dma_start(wq_sb[:, 0:512], wq_ext[:, 0:512])
            nc.sync.dma_start(wq_sb[:, 512:1024], wq_ext[:, 512:1024])
            nc.sync.dma_start(wk_sb[:], wk_ext[:])
            nc.sync.dma_start(wv_sb[:], wv_ext[:])
            for tcn in range(1, 4):
                for ec in range(0, EC, 2):
                    nc.sync.dma_start(
                        x_sb[tcn][:, ec, :],
                        xT_ext[ec * 128 : (ec + 1) * 128, tcn * 512 : (tcn + 1) * 512],
                    )
            for tcn in range(4):
                for ec in range(1, EC, 2):
                    nc.scalar.dma_start(
                        x_sb[tcn][:, ec, :],
                        xT_ext[ec * 128 : (ec + 1) * 128, tcn * 512 : (tcn + 1) * 512],
                    )

            # ---------------- qkv emission helpers ----------------
            def emit_QK(t, wsb, dst):
                ps = psum.tile([128, 1024], F32, tag="spair", bufs=3)
                for ec in range(EC):
                    nc.tensor.matmul(
                        ps[:, 0:512],
                        wsb[:, ec * 128 : (ec + 1) * 128],
                        x_sb[t][:, ec, :],
                        start=(ec == 0),
                        stop=(ec == EC - 1),
                    )
                nc.vector.tensor_copy(dst[:, t * 512 : (t + 1) * 512], ps[:, 0:512])

            def emit_Q(t):
                emit_QK(t, wq_sb, qt_sb)

            def emit_K(t):
                emit_QK(t, wk_sb, kt_sb)

            class FeedItem:
                """A qkv feeder work unit split in two PE half-bursts so its
                matmuls interleave with the slot's PV pair — the halves'
                LDWEIGHTS then load in the background of the long PV/scores
                streams instead of serializing (V production is otherwise
                LDWEIGHTS-bound: 256 loads x ~107ns)."""

                def __init__(self, kind, t, tsub=0):
                    self.kind, self.t, self.tsub = kind, t, tsub
                    self.ps = None

                def _mm(self, ec):
                    t, tsub = self.t, self.tsub
                    if self.kind == "V":
                        nc.tensor.matmul(
                            self.ps[:, 0:128],
                            x_sb[t][:, ec, tsub * 128 : (tsub + 1) * 128],
                            wv_sb[:, ec * 128 : (ec + 1) * 128],
                            start=(ec == 0),
                            stop=(ec == EC - 1),
                        )
                    else:
                        wsb = wq_sb if self.kind == "Q" else wk_sb
                        nc.tensor.matmul(
                            self.ps[:, 0:512],
                            wsb[:, ec * 128 : (ec + 1) * 128],
                            x_sb[t][:, ec, :],
                            start=(ec == 0),
                            stop=(ec == EC - 1),
                        )

                def half1(self):
                    self.ps = psum.tile([128, 1024], F32, tag="spair", bufs=3)
                    for ec in range(4):
                        self._mm(ec)

                def half2(self):
                    for ec in range(4, EC):
                        self._mm(ec)

                def copies(self):
                    t = self.t
                    if self.kind == "V":
                        g = t * 4 + self.tsub
                        nc.vector.tensor_copy(vones[:, g, 64:128], self.ps[:, 0:64])
                        nc.vector.tensor_copy(
                            vones[:, g, 192:256], self.ps[:, 64:128]
                        )
                    else:
                        dst = qt_sb if self.kind == "Q" else kt_sb
                        nc.vector.tensor_copy(
                            dst[:, t * 512 : (t + 1) * 512], self.ps[:, 0:512]
                        )

                def emit_all(self):
                    self.half1()
                    self.half2()
                    self.copies()

            def V(t, tsub):
                return ("V", t, tsub)

            def K(t):
                return ("K", t)

            def Q(t):
                return ("Q", t)

            # ---------------- prologue: minimal batch-0 qkv ----------------
            # Just enough for the exp stream to start: Q/K of the first tile
            # (K1 covers the depth-2 prefetch of kb=4 at slot 2).
            emit_Q(0)
            emit_K(0)
            emit_K(1)

            # feeder schedule: (iter, kb) -> one work unit (a second unit on
            # a few slots is emitted un-interleaved after the PV pair).
            # Deadlines (depth-2 scores prefetch): K-block m is needed when
            # scores(kb=m) is prefetched at slot m-2; V-block m when PV(kb=m)
            # runs at slot m; Q(tcn) two slots before its iteration starts.
            FEED = {
                (0, 1): [V(0, 2)],
                (0, 2): [V(0, 3)],
                (0, 3): [V(1, 0), K(2)],
                (0, 4): [V(1, 1)],
                (0, 5): [V(1, 2)],
                (0, 6): [V(1, 3)],
                (0, 7): [V(2, 0), K(3)],
                (0, 8): [V(2, 1)],
                (0, 9): [V(2, 2)],
                (0, 10): [V(2, 3)],
                (0, 11): [V(3, 0), Q(1)],
                (0, 12): [V(3, 1)],
                (0, 13): [V(3, 2)],
                (0, 14): [V(3, 3)],
                (1, 0): [Q(2)],
                (1, 2): [K(4)],
                (1, 4): [V(4, 0)],
                (1, 6): [V(4, 1)],
                (1, 8): [V(4, 2)],
                (1, 10): [V(4, 3)],
                (1, 12): [Q(3)],
                (2, 0): [K(5)],
                (2, 2): [V(5, 0)],
                (2, 4): [V(5, 1)],
                (2, 6): [V(5, 2)],
                (2, 8): [V(5, 3)],
                (2, 10): [K(6)],
                (3, 0): [V(6, 0)],
                (3, 2): [V(6, 1)],
                (3, 4): [V(6, 2)],
                (3, 6): [V(6, 3)],
                (3, 8): [K(7)],
                (3, 10): [Q(4)],
                (4, 0): [V(7, 0)],
                (4, 2): [V(7, 1)],
                (4, 4): [V(7, 2)],
                (4, 6): [V(7, 3)],
                (4, 10): [Q(5)],
                (5, 2): [Q(6)],
                (5, 8): [Q(7)],
            }
            # x(b1) chunk DMAs issued on sync from iters 0-1: (iter, kb) -> tcn
            XB1 = {}
            for i, t in enumerate(range(4, 8)):
                for ec in range(EC):
                    slot = i * EC + ec  # 0..31 over iters 0-1
                    XB1.setdefault((slot // 16, slot % 16), []).append((t, ec))

            # ---------------- attention ----------------
            a2a_in = dram.tile([1024, 512], BF)
            a2a_out = dram.tile([1024, 512], BF)
            warm_in = dram.tile([1, 512], BF)
            warm_out = dram.tile([8, 512], BF)

            def emit_norm_head(pend, j, rec_in):
                """Normalize one head of a finished (b, qb) iteration's raw
                attention output; overlapped with the next iteration."""
                pb, pqb, raws, _den = pend
                # broadcast 1/denom to partitions 64..127 (col-tiled M=64
                # matmul so the tile is a single psum bank); head B's recip
                # row lives on partition 32 to stay partition-aligned.
                p0 = 32 * j
                bcp = psum.tile([128, 512], F32, tag="spair", bufs=3)
                nc.tensor.matmul(
                    bcp[64:128, :],
                    bones[p0 : p0 + 1, 0:64],
                    rec_in[p0 : p0 + 1, :],
                    start=True,
                    stop=True,
                )
                onorm = npool.tile([128, 512], BF, tag="onorm")
                nc.vector.tensor_mul(
                    onorm[64:128, :], raws[j][64:128, :], bcp[64:128, :]
                )
                row = 128 * (4 * pb + pqb) + 64 * j
                nc.sync.dma_start(a2a_in[row : row + 64, :], onorm[64:128, :])

            def emit_scores(b, qb, kb):
                qoff = b * N + qb * 512
                koff = b * N + kb * 128
                sp = psum.tile([128, 1024], F32, tag="spair", bufs=3)
                nc.tensor.matmul(
                    sp[:, 0:512],
                    kt_sb[0:64, koff : koff + 128],
                    qt_sb[0:64, qoff : qoff + 512],
                    start=True,
                    stop=True,
                )
                nc.tensor.matmul(
                    sp[:, 512:1024],
                    kt_sb[64:128, koff : koff + 128],
                    qt_sb[64:128, qoff : qoff + 512],
                    start=True,
                    stop=True,
                )
                e_t = epool.tile([128, 1024], BF)
                nc.scalar.activation(e_t[:], sp[:], AF.Exp, scale=SCALE)
                return e_t

            iters = [(b, qb) for b in range(B) for qb in range(N // 512)]
            NBLK = len(iters) * 16  # 128 half-block positions
            pending = None
            rec_cur = None
            warm_src = None
            e_tiles = {}
            # depth-2 prefetch: two scores blocks always in flight ahead of
            # the exp being consumed, so feeder bursts can't starve ACT
            e_tiles[0] = emit_scores(*iters[0], 0)
            e_tiles[1] = emit_scores(*iters[0], 1)
            oA = oB = None
            for p in range(NBLK):
                it_idx, kb = divmod(p, 16)
                b, qb = iters[it_idx]
                g = b * (N // 128) + kb
                if kb == 0:
                    oA = psum.tile([128, 512], F32, tag="oA", bufs=1)
                    oB = psum.tile([128, 512], F32, tag="oB", bufs=1)
                if p + 2 < NBLK:
                    it2, kb2 = divmod(p + 2, 16)
                    e_tiles[p + 2] = emit_scores(*iters[it2], kb2)
                e_t = e_tiles.pop(p)
                last = kb == 15
                items = [FeedItem(*x) for x in FEED.get((it_idx, kb), ())]
                if p == 0:
                    # first two V blocks must precede the very first PV
                    FeedItem("V", 0, 0).emit_all()
                    FeedItem("V", 0, 1).emit_all()
                # first feeder item's matmul halves go between the PV pair:
                # their weight loads hide under the PV streams
                if items:
                    items[0].half1()
                nc.tensor.matmul(
                    oA[:], vones[:, g, 0:128], e_t[:, 0:512],
                    start=(kb == 0), stop=last,
                )
                if items:
                    items[0].half2()
                nc.tensor.matmul(
                    oB[:], vones[:, g, 128:256], e_t[:, 512:1024],
                    start=(kb == 0), stop=last,
                )
                if items:
                    items[0].copies()
                    for extra in items[1:]:
                        extra.emit_all()
                # x(b1) DMA issues + norms of the previous iteration
                for (t, ec) in XB1.get((it_idx, kb), ()):
                    nc.sync.dma_start(
                        x_sb[t][:, ec, :],
                        xT_ext[ec * 128 : (ec + 1) * 128, t * 512 : (t + 1) * 512],
                    )
                if (it_idx, kb) == (5, 0):
                    # proj weights: issued mid-attention when HBM is idle
                    nc.sync.dma_start(wp_sb[:], wp_ext[:])
                    nc.sync.dma_start(bias_sb[:], bias_ext[:])
                if kb == 3 and pending is not None:
                    # one reciprocal covers both heads (dens on partitions
                    # 0 and 32)
                    rec_cur = npool.tile([33, 512], BF, tag="recb", bufs=2)
                    with nc.allow_low_precision(reason="bf16 softmax 1/denom"):
                        nc.vector.reciprocal(rec_cur[:], pending[3][:])
                if kb == 7 and pending is not None:
                    emit_norm_head(pending, 0, rec_cur)
                if kb == 10 and it_idx == 6 and pending is not None:
                    # keep a late-written tile as the warm-collective DMA
                    # source so the scheduler cannot hoist the ncfw warm-up
                    warm_src = pending[2][0]
                if kb == 11 and pending is not None:
                    emit_norm_head(pending, 1, rec_cur)
                    pending = None
                if kb == 13 and it_idx == 6 and warm_src is not None:
                    nc.sync.dma_start(warm_in[:], warm_src[64:65, 0:512])
                    nc.gpsimd.collective_compute(
                        "AllGather",
                        ALU.bypass,
                        ins=[warm_in.opt()],
                        outs=[warm_out.opt()],
                        replica_groups=[list(range(NCORES))],
                    )
                if last:
                    # stash raw output + denominators in SBUF so the psum
                    # accumulators free; ordered oA-first (den A then raw A)
                    # so the next iteration's PV can restart on oA after two
                    # DVE ops instead of four. On the final iteration the
                    # raw copies run on the now-idle Scalar engine so the
                    # DVE can start the reciprocal immediately.
                    den = npool.tile([33, 512], F32, tag="den", bufs=3)
                    raws = []
                    for j, oX in ((0, oA), (1, oB)):
                        nc.vector.tensor_copy(
                            den[32 * j : 32 * j + 1, :], oX[32 * j : 32 * j + 1, :]
                        )
                        raw = npool.tile([128, 512], BF, tag=f"raw{j}", bufs=3)
                        if p == NBLK - 1:
                            nc.scalar.copy(raw[64:128, :], oX[64:128, :])
                        else:
                            nc.vector.tensor_copy(raw[64:128, :], oX[64:128, :])
                        raws.append(raw)
                    pending = (b, qb, raws, den)

            # tail: one reciprocal for the last iteration, then both norms
            rec_tail = npool.tile([33, 512], BF, tag="recb", bufs=2)
            with nc.allow_low_precision(reason="bf16 softmax 1/denom"):
                nc.vector.reciprocal(rec_tail[:], pending[3][:])
            emit_norm_head(pending, 0, rec_tail)
            emit_norm_head(pending, 1, rec_tail)

            nc.gpsimd.collective_compute(
                "AllToAll",
                ALU.bypass,
                ins=[a2a_in.opt()],
                outs=[a2a_out.opt()],
                replica_groups=[list(range(NCORES))],
            )

            # paced dummy-matmul chain: keeps the PE's activity monitor at
            # full clock across the AllToAll so proj doesn't run at 1.2GHz.
            # Three matmuls per link (~80% PE duty) paced by a small DVE
            # read-back; the light single-matmul tail links drain fast once
            # the collective lands so they don't delay proj.
            for link in range(32):
                heavy = link < 24
                dps = psum.tile([128, 1024], F32, tag="spair", bufs=3)
                nc.tensor.matmul(
                    dps[:, 0:512], garb[:, 0:128], garb[:, 0:512],
                    start=True, stop=True,
                )
                if heavy:
                    nc.tensor.matmul(
                        dps[:, 512:1024], garb[:, 0:128], garb[:, 0:512],
                        start=True, stop=True,
                    )
                    nc.tensor.matmul(
                        dps[:, 0:512], garb[:, 0:128], garb[:, 0:512],
                        start=True, stop=True,
                    )
                nc.vector.tensor_copy(gout[:, 0:64], dps[:, 0:64])

            # ---------------- proj ----------------
            rhs_sb = cpool.tile([128, EC, 512], BF)
            for kc in range(EC):
                nc.sync.dma_start(
                    rhs_sb[:, kc, :], a2a_out[kc * 128 : (kc + 1) * 128, :]
                )
            for ecn in range(EC):
                yp = psum.tile([128, 1024], F32, tag="spair", bufs=3)
                for kc in range(EC):
                    nc.tensor.matmul(
                        yp[:, 0:512],
                        wp_sb[:, kc * 1024 + ecn * 128 : kc * 1024 + (ecn + 1) * 128],
                        rhs_sb[:, kc, :],
                        start=(kc == 0),
                        stop=(kc == EC - 1),
                    )
                y_sb = ypool.tile([128, 512], F32)
                nc.vector.tensor_scalar(
                    out=y_sb[:],
                    in0=yp[:, 0:512],
                    scalar1=bias_sb[:, ecn : ecn + 1],
                    scalar2=None,
                    op0=ALU.add,
                )
                nc.sync.dma_start(out_ext[ecn * 128 : (ecn + 1) * 128, :], y_sb[:])

    _split_multi_waits(nc)
    return nc


def _make_in_maps(x, w_qkv, w_proj, b_proj):
    x = np.asarray(x, dtype=np.float32)
    w_qkv = np.asarray(w_qkv, dtype=np.float32)
    w_proj = np.asarray(w_proj, dtype=np.float32)
    b_proj = np.asarray(b_proj, dtype=np.float32)

    xT = np.ascontiguousarray(x.reshape(TOK, D).T).astype(BF16)
    wq_full = w_qkv[:, 0:D]
    wk_full = w_qkv[:, D : 2 * D]
    wv_full = w_qkv[:, 2 * D : 3 * D]

    def to_sb(wpair):  # [1024, 128] -> [128, 8*128] (e-chunk-major columns)
        return np.ascontiguousarray(
            wpair.reshape(EC, 128, 128).transpose(1, 0, 2).reshape(128, 1024)
        ).astype(BF16)

    wp_sb = np.ascontiguousarray(
        w_proj.reshape(EC, 128, 1024).transpose(1, 0, 2).reshape(128, 8192)
    ).astype(BF16)
    bias_sb = np.ascontiguousarray(b_proj.reshape(EC, 128).T).astype(np.float32)

    in_maps = []
    for c in range(NCORES):
        hA, hB = 2 * c, 2 * c + 1

        def pair(w):
            return np.concatenate(
                [w[:, hA * HD : (hA + 1) * HD], w[:, hB * HD : (hB + 1) * HD]], axis=1
            )

        in_maps.append(
            {
                "xT": xT,
                "wq": to_sb(pair(wq_full)),
                "wk": to_sb(pair(wk_full)),
                "wv": to_sb(pair(wv_full)),
                "wp": wp_sb,
                "bias": bias_sb,
                "coreid": np.array([[c]], dtype=np.uint32),
            }
        )
    return in_maps


_CACHE = {}


def kernel(x, w_qkv, w_proj, b_proj):
    import concourse.bass_utils as bass_utils

    bass_utils.upload_artifacts = lambda tmpdir: tmpdir  # no S3 in container

    if "nc" not in _CACHE:
        _CACHE["nc"] = _build_nc()
    nc = _CACHE["nc"]

    in_maps = _make_in_maps(x, w_qkv, w_proj, b_proj)

    trace = _install_axon_profile_hook()
    try:
        res = bass_utils.run_bass_kernel_spmd(
            nc, in_maps, list(range(NCORES)), trace=trace
        )
    except Exception:
        if not trace:
            raise
        res = bass_utils.run_bass_kernel_spmd(
            nc, in_maps, list(range(NCORES)), trace=False
        )

    kernel.last_exec_time_ns = res.exec_time_ns

    out = np.empty((B, N, D), dtype=np.float32)
    for c in range(NCORES):
        yT = np.asarray(res.results[c]["out"], dtype=np.float32)  # [1024, 512]
        b, s = c // 4, c % 4
        out[b, s * 512 : (s + 1) * 512, :] = yT.T
    return out


kernel.last_exec_time_ns = None


# revision 16
# speedup vs baseline: 1.1150x; 1.1150x over previous
"""Distributed Trainium2 kernel for nn_Attention (B=2, N=2048, D=1024, H=16).

Sharding: tensor-parallel over heads (2 heads per core) for qkv + attention,
then an AllToAll redistributes attention output so each core projects a
512-row slice of the output (cores 0-3: batch 0, cores 4-7: batch 1).

v2 restructure (from trace analysis of the 282us baseline):
  - The attention phase is ACT(exp)-bound (~1126ns per [128,1024] block,
    back-to-back); the PE has ~40% slack there but the baseline ran qkv and
    proj as separate serial phases. Now only batch-0's minimal qkv prefix
    (Q0,K0,V0,K1,V1) runs as a prologue; every remaining qkv tile-group is
    fed into the attention iterations' PE slack via a (iter,kb) schedule
    that respects just-in-time deadlines (K/V blocks consumed at kb pace).
  - HAM kept warm: the PE's ~50% duty cycle let the activity monitor
    re-throttle it to 1.2GHz for 35..204us of the baseline (all attention
    matmuls ran 433ns instead of 216ns). Interleaved qkv raises duty in
    iters 0-4; a paced dummy-matmul chain bridges the AllToAll so proj
    starts at 2.4GHz instead of half clock.
  - AllToAll warm-up collective is pinned late (its input DMA sources
    iteration-6 data) instead of being hoisted to t=72us by the scheduler.
  - Both heads' softmax denominators land on psum partitions 0/32 (head-B
    ones column moved), so one DVE reciprocal [33,512] covers both heads
    (iterative divide is per-free-element: halves 6.6us/iter -> 3.3).
  - Per-core dataflow otherwise as baseline: Q^T/K^T [128, 4096] bf16,
    V packed in vones with ones-columns so PV also produces denominators;
    scores per 128k x 512q with two heads row-tiled concurrently on the PE;
    exp on ScalarE (no max subtraction; |s|<~7); deferred normalization.
"""

import sys
import types

import numpy as np

if "/opt/trn_rl_repo" not in sys.path:
    sys.path.insert(0, "/opt/trn_rl_repo")

import ml_dtypes

B, N, D = 2, 2048, 1024
H, HD = 16, 64
SCALE = HD**-0.5
TOK = B * N  # 4096, token index = b*N + t
EC = 8  # embed-dim chunks of 128
NCORES = 8
# per k-block vones layout [128 tok, 256]:
#   [1 | 0*63 | V_A(64) | 0*32 | 1 | 0*31 | V_B(64)]
# so the PV matmul (M=128) puts head-A's softmax denominator on psum
# partition 0 and head-B's on partition 32 (32-aligned so the DVE den
# copies stay partition-aligned and one [33,512] reciprocal covers both).
VSTRIDE = 256
NKB = TOK // 128  # 32 k-blocks across both batches
NTCN = TOK // 512  # 8 qkv token tiles

BF16 = ml_dtypes.bfloat16


def _install_axon_profile_hook():
    """Best-effort: register the NTFF profile hook the RL container's antenv
    stub omits, so run_bass_kernel_spmd(trace=True) can report exec_time_ns."""
    try:
        import antenv

        if "antenv.axon_hooks" not in sys.modules:
            hooks = types.ModuleType("antenv.axon_hooks")
            hooks._hook = None
            hooks.set_axon_ntff_profile_hook = lambda h: setattr(hooks, "_hook", h)
            hooks.get_axon_ntff_profile_hook = lambda: hooks._hook
            sys.modules["antenv.axon_hooks"] = hooks
            antenv.axon_hooks = hooks
            from trn_agent_boot.trn_boot import _ntff_profile_via_ctypes

            hooks.set_axon_ntff_profile_hook(
                _ntff_profile_via_ctypes("/opt/axon/libaxon_pjrt.so")
            )
        return True
    except Exception:
        return False


def _split_multi_waits(nc):
    """neuronxcc's walrus (CoreV3 setupSyncWait) rejects instructions that
    carry more than one semaphore wait, but Tile's wait assignment freely
    attaches several. Hoist the extra waits onto freshly inserted same-engine
    NoOps placed directly before the instruction — the engine stalls at the
    same program point, so semantics are unchanged."""
    import concourse.mybir as mybir

    n_split = 0
    for fn in nc.m.functions:
        for bb in fn.blocks:
            insts = bb.instructions
            if not any(
                i.sync_info is not None and len(i.sync_info.on_wait) > 1
                for i in insts
            ):
                continue
            new_insts = []
            for ins in insts:
                si = ins.sync_info
                if si is not None and len(si.on_wait) > 1:
                    waits = list(si.on_wait)
                    for w in waits[:-1]:
                        nop = mybir.InstNoOp(
                            name=f"wsplit-{n_split}",
                            engine=ins.engine,
                            ins=[],
                            outs=[],
                            sync_info=mybir.SyncInfo(on_wait=[w], on_update=[]),
                        )
                        new_insts.append(nop)
                        n_split += 1
                    ins.sync_info = mybir.SyncInfo(
                        on_wait=[waits[-1]], on_update=list(si.on_update)
                    )
                new_insts.append(ins)
            bb.instructions = new_insts


def _build_nc():
    import concourse.bass as bass
    import concourse.mybir as mybir
    import concourse.tile as tile

    F32 = mybir.dt.float32
    BF = mybir.dt.bfloat16
    AF = mybir.ActivationFunctionType
    ALU = mybir.AluOpType

    nc = bass.Bass()
    xT_ext = nc.declare_dram_parameter("xT", [D, TOK], BF, isOutput=False)
    wq_ext = nc.declare_dram_parameter("wq", [128, 1024], BF, isOutput=False)
    wk_ext = nc.declare_dram_parameter("wk", [128, 1024], BF, isOutput=False)
    wv_ext = nc.declare_dram_parameter("wv", [128, 1024], BF, isOutput=False)
    wp_ext = nc.declare_dram_parameter("wp", [128, 8192], BF, isOutput=False)
    bias_ext = nc.declare_dram_parameter("bias", [128, 8], F32, isOutput=False)
    coreid_ext = nc.declare_dram_parameter(
        "coreid", [1, 1], mybir.dt.uint32, isOutput=False
    )
    out_ext = nc.declare_dram_parameter("out", [D, 512], F32, isOutput=True)

    with tile.TileContext(nc) as tc:
        with (
            tc.tile_pool(name="const", bufs=1) as cpool,
            tc.tile_pool(name="e", bufs=6) as epool,
            tc.tile_pool(name="norm", bufs=2) as npool,
            tc.tile_pool(name="y", bufs=2) as ypool,
            tc.tile_pool(name="psum", bufs=2, space="PSUM") as psum,
            tc.tile_pool(name="dram", bufs=1, space="DRAM") as dram,
        ):
            wq_sb = cpool.tile([128, 1024], BF)
            wk_sb = cpool.tile([128, 1024], BF)
            wv_sb = cpool.tile([128, 1024], BF)
            wp_sb = cpool.tile([128, 8192], BF)
            bias_sb = cpool.tile([128, 8], F32)
            qt_sb = cpool.tile([128, TOK], BF)
            kt_sb = cpool.tile([128, TOK], BF)
            vones = cpool.tile([128, NKB, VSTRIDE], BF)
            # all 8 x token-tiles stay resident (qkv is spread over attention)
            x_sb = [
                cpool.tile([128, EC, 512], BF, name=f"x{t}") for t in range(NTCN)
            ]
            # garbage tiles for HAM keep-warm dummy matmuls (never written;
            # NaN results land in recycled psum slots and are never read)
            garb = cpool.tile([128, 512], BF)
            gout = cpool.tile([128, 512], BF)

            nc.vector.memset(garb[:], 0.0)
            nc.vector.memset(vones[:], 0.0)
            nc.vector.memset(vones[:, :, 0:1], 1.0)
            nc.vector.memset(vones[:, :, 160:161], 1.0)
            bones = cpool.tile([33, 64], BF)
            nc.vector.memset(bones[0:1, :], 1.0)
            nc.vector.memset(bones[32:33, :], 1.0)

            # ---------------- input DMA issue schedule ----------------
            # sync gets the weights + even x(b0) chunks; scalar (idle until
            # the first exp) gets the odd chunks. x(b1) is issued from
            # inside attention iterations 0-1 on sync; wp/bias mid-attention
            # (the prologue window is HBM-bandwidth limited: 8 cores pull
            # replicated x concurrently).
            for ec in range(0, EC, 2):
                nc.sync.dma_start(
                    x_sb[0][:, ec, :], xT_ext[ec * 128 : (ec + 1) * 128, 0:512]
                )
            nc.sync.dma_start(wq_sb[:, 0:512], wq_ext[:, 0:512])
            nc.sync.dma_start(wq_sb[:, 512:1024], wq_ext[:, 512:1024])
            nc.sync.dma_start(wk_sb[:], wk_ext[:])
            nc.sync.dma_start(wv_sb[:], wv_ext[:])
            for tcn in range(1, 4):
                for ec in range(0, EC, 2):
                    nc.sync.dma_start(
                        x_sb[tcn][:, ec, :],
                        xT_ext[ec * 128 : (ec + 1) * 128, tcn * 512 : (tcn + 1) * 512],
                    )
            for tcn in range(4):
                for ec in range(1, EC, 2):
                    nc.scalar.dma_start(
                        x_sb[tcn][:, ec, :],
                        xT_ext[ec * 128 : (ec + 1) * 128, tcn * 512 : (tcn + 1) * 512],
                    )

            # ---------------- qkv emission helpers ----------------
            def emit_QK(t, wsb, dst):
                ps = psum.tile([128, 1024], F32, tag="spair", bufs=3)
                for ec in range(EC):
                    nc.tensor.matmul(
                        ps[:, 0:512],
                        wsb[:, ec * 128 : (ec + 1) * 128],
                        x_sb[t][:, ec, :],
                        start=(ec == 0),
                        stop=(ec == EC - 1),
                    )
                nc.vector.tensor_copy(dst[:, t * 512 : (t + 1) * 512], ps[:, 0:512])

            def emit_Q(t):
                emit_QK(t, wq_sb, qt_sb)

            def emit_K(t):
                emit_QK(t, wk_sb, kt_sb)

            class FeedItem:
                """A qkv feeder work unit split in two PE half-bursts so its
                matmuls interleave with the slot's PV pair — the halves'
                LDWEIGHTS then load in the background of the long PV/scores
                streams instead of serializing (V production is otherwise
                LDWEIGHTS-bound: 256 loads x ~107ns)."""

                def __init__(self, kind, t, tsub=0):
                    self.kind, self.t, self.tsub = kind, t, tsub
                    self.ps = None

                def _mm(self, ec):
                    t, tsub = self.t, self.tsub
                    if self.kind == "V":
                        nc.tensor.matmul(
                            self.ps[:, 0:128],
                            x_sb[t][:, ec, tsub * 128 : (tsub + 1) * 128],
                            wv_sb[:, ec * 128 : (ec + 1) * 128],
                            start=(ec == 0),
                            stop=(ec == EC - 1),
                        )
                    else:
                        wsb = wq_sb if self.kind == "Q" else wk_sb
                        nc.tensor.matmul(
                            self.ps[:, 0:512],
                            wsb[:, ec * 128 : (ec + 1) * 128],
                            x_sb[t][:, ec, :],
                            start=(ec == 0),
                            stop=(ec == EC - 1),
                        )

                def half1(self):
                    self.ps = psum.tile([128, 1024], F32, tag="spair", bufs=3)
                    for ec in range(4):
                        self._mm(ec)

                def half2(self):
                    for ec in range(4, EC):
                        self._mm(ec)

                def copies(self):
                    t = self.t
                    if self.kind == "V":
                        g = t * 4 + self.tsub
                        nc.vector.tensor_copy(vones[:, g, 64:128], self.ps[:, 0:64])
                        nc.vector.tensor_copy(
                            vones[:, g, 192:256], self.ps[:, 64:128]
                        )
                    else:
                        dst = qt_sb if self.kind == "Q" else kt_sb
                        nc.vector.tensor_copy(
                            dst[:, t * 512 : (t + 1) * 512], self.ps[:, 0:512]
                        )

                def emit_all(self):
                    self.half1()
                    self.half2()
                    self.copies()

            def V(t, tsub):
                return ("V", t, tsub)

            def K(t):
                return ("K", t)

            def Q(t):
                return ("Q", t)

            # ---------------- prologue: minimal batch-0 qkv ----------------
            # Just enough for the exp stream to start: Q/K of the first tile
            # (K1 covers the depth-2 prefetch of kb=4 at slot 2).
            emit_Q(0)
            emit_K(0)
            emit_K(1)

            # feeder schedule: (iter, kb) -> one work unit (a second unit on
            # a few slots is emitted un-interleaved after the PV pair).
            # Deadlines (depth-2 scores prefetch): K-block m is needed when
            # scores(kb=m) is prefetched at slot m-2; V-block m when PV(kb=m)
            # runs at slot m; Q(tcn) two slots before its iteration starts.
            FEED = {
                (0, 1): [V(0, 2)],
                (0, 2): [V(0, 3)],
                (0, 3): [V(1, 0), K(2)],
                (0, 4): [V(1, 1)],
                (0, 5): [V(1, 2)],
                (0, 6): [V(1, 3)],
                (0, 7): [V(2, 0), K(3)],
                (0, 8): [V(2, 1)],
                (0, 9): [V(2, 2)],
                (0, 10): [V(2, 3)],
                (0, 11): [V(3, 0), Q(1)],
                (0, 12): [V(3, 1)],
                (0, 13): [V(3, 2)],
                (0, 14): [V(3, 3)],
                (1, 0): [Q(2)],
                (1, 2): [K(4)],
                (1, 4): [V(4, 0)],
                (1, 6): [V(4, 1)],
                (1, 8): [V(4, 2)],
                (1, 10): [V(4, 3)],
                (1, 12): [Q(3)],
                (2, 0): [K(5)],
                (2, 2): [V(5, 0)],
                (2, 4): [V(5, 1)],
                (2, 6): [V(5, 2)],
                (2, 8): [V(5, 3)],
                (2, 10): [K(6)],
                (3, 0): [V(6, 0)],
                (3, 2): [V(6, 1)],
                (3, 4): [V(6, 2)],
                (3, 6): [V(6, 3)],
                (3, 8): [K(7)],
                (3, 10): [Q(4)],
                (4, 0): [V(7, 0)],
                (4, 2): [V(7, 1)],
                (4, 4): [V(7, 2)],
                (4, 6): [V(7, 3)],
                (4, 10): [Q(5)],
                (5, 2): [Q(6)],
                (5, 8): [Q(7)],
            }
            # x(b1) chunk DMAs issued on sync from iters 0-1: (iter, kb) -> tcn
            XB1 = {}
            for i, t in enumerate(range(4, 8)):
                for ec in range(EC):
                    slot = i * EC + ec  # 0..31 over iters 0-1
                    XB1.setdefault((slot // 16, slot % 16), []).append((t, ec))

            # ---------------- attention ----------------
            a2a_in = dram.tile([1024, 512], BF)
            a2a_out = dram.tile([1024, 512], BF)
            warm_in = dram.tile([1, 512], BF)
            warm_out = dram.tile([8, 512], BF)

            def emit_norm_head(pend, j, rec_in):
                """Normalize one head of a finished (b, qb) iteration's raw
                attention output; overlapped with the next iteration."""
                pb, pqb, raws, _den = pend
                # broadcast 1/denom to partitions 64..127 (col-tiled M=64
                # matmul so the tile is a single psum bank); head B's recip
                # row lives on partition 32 to stay partition-aligned.
                p0 = 32 * j
                bcp = psum.tile([128, 512], F32, tag="spair", bufs=3)
                nc.tensor.matmul(
                    bcp[64:128, :],
                    bones[p0 : p0 + 1, 0:64],
                    rec_in[p0 : p0 + 1, :],
                    start=True,
                    stop=True,
                )
                onorm = npool.tile([128, 512], BF, tag="onorm")
                nc.vector.tensor_mul(
                    onorm[64:128, :], raws[j][64:128, :], bcp[64:128, :]
                )
                row = 128 * (4 * pb + pqb) + 64 * j
                nc.sync.dma_start(a2a_in[row : row + 64, :], onorm[64:128, :])

            def emit_scores(b, qb, kb):
                qoff = b * N + qb * 512
                koff = b * N + kb * 128
                sp = psum.tile([128, 1024], F32, tag="spair", bufs=3)
                nc.tensor.matmul(
                    sp[:, 0:512],
                    kt_sb[0:64, koff : koff + 128],
                    qt_sb[0:64, qoff : qoff + 512],
                    start=True,
                    stop=True,
                )
                nc.tensor.matmul(
                    sp[:, 512:1024],
                    kt_sb[64:128, koff : koff + 128],
                    qt_sb[64:128, qoff : qoff + 512],
                    start=True,
                    stop=True,
                )
                e_t = epool.tile([128, 1024], BF)
                nc.scalar.activation(e_t[:], sp[:], AF.Exp, scale=SCALE)
                return e_t

            iters = [(b, qb) for b in range(B) for qb in range(N // 512)]
            NBLK = len(iters) * 16  # 128 half-block positions
            pending = None
            rec_cur = None
            warm_src = None
            e_tiles = {}
            # depth-2 prefetch: two scores blocks always in flight ahead of
            # the exp being consumed, so feeder bursts can't starve ACT
            e_tiles[0] = emit_scores(*iters[0], 0)
            e_tiles[1] = emit_scores(*iters[0], 1)
            oA = oB = None
            for p in range(NBLK):
                it_idx, kb = divmod(p, 16)
                b, qb = iters[it_idx]
                g = b * (N // 128) + kb
                if kb == 0:
                    oA = psum.tile([128, 512], F32, tag="oA", bufs=1)
                    oB = psum.tile([128, 512], F32, tag="oB", bufs=1)
                if p + 2 < NBLK:
                    it2, kb2 = divmod(p + 2, 16)
                    e_tiles[p + 2] = emit_scores(*iters[it2], kb2)
                e_t = e_tiles.pop(p)
                last = kb == 15
                items = [FeedItem(*x) for x in FEED.get((it_idx, kb), ())]
                if p == 0:
                    # first two V blocks must precede the very first PV
                    FeedItem("V", 0, 0).emit_all()
                    FeedItem("V", 0, 1).emit_all()
                nc.tensor.matmul(
                    oA[:], vones[:, g, 0:128], e_t[:, 0:512],
                    start=(kb == 0), stop=last,
                )
                nc.tensor.matmul(
                    oB[:], vones[:, g, 128:256], e_t[:, 512:1024],
                    start=(kb == 0), stop=last,
                )
                # feeder items trail the PV pair; interleaving them between
                # the PV matmuls measurably slows the ACT stream (~20%),
                # so they stay as trailing bursts bridged by the depth-2
                # scores prefetch
                for item in items:
                    item.emit_all()
                # x(b1) DMA issues + norms of the previous iteration
                for (t, ec) in XB1.get((it_idx, kb), ()):
                    nc.sync.dma_start(
                        x_sb[t][:, ec, :],
                        xT_ext[ec * 128 : (ec + 1) * 128, t * 512 : (t + 1) * 512],
                    )
                if (it_idx, kb) == (5, 0):
                    # proj weights: issued mid-attention when HBM is idle
                    nc.sync.dma_start(wp_sb[:], wp_ext[:])
                    nc.sync.dma_start(bias_sb[:], bias_ext[:])
                if kb == 3 and pending is not None:
                    # one reciprocal covers both heads (dens on partitions
                    # 0 and 32)
                    rec_cur = npool.tile([33, 512], BF, tag="recb", bufs=2)
                    with nc.allow_low_precision(reason="bf16 softmax 1/denom"):
                        nc.vector.reciprocal(rec_cur[:], pending[3][:])
                if kb == 7 and pending is not None:
                    emit_norm_head(pending, 0, rec_cur)
                if kb == 10 and it_idx == 6 and pending is not None:
                    # keep a late-written tile as the warm-collective DMA
                    # source so the scheduler cannot hoist the ncfw warm-up
                    warm_src = pending[2][0]
                if kb == 11 and pending is not None:
                    emit_norm_head(pending, 1, rec_cur)
                    pending = None
                if kb == 13 and it_idx == 6 and warm_src is not None:
                    nc.sync.dma_start(warm_in[:], warm_src[64:65, 0:512])
                    nc.gpsimd.collective_compute(
                        "AllGather",
                        ALU.bypass,
                        ins=[warm_in.opt()],
                        outs=[warm_out.opt()],
                        replica_groups=[list(range(NCORES))],
                    )
                if last:
                    # stash raw output + denominators in SBUF so the psum
                    # accumulators free; ordered oA-first (den A then raw A)
                    # so the next iteration's PV can restart on oA after two
                    # DVE ops instead of four. On the final iteration the
                    # raw copies run on the now-idle Scalar engine so the
                    # DVE can start the reciprocal immediately.
                    den = npool.tile([33, 512], F32, tag="den", bufs=3)
                    raws = []
                    for j, oX in ((0, oA), (1, oB)):
                        nc.vector.tensor_copy(
                            den[32 * j : 32 * j + 1, :], oX[32 * j : 32 * j + 1, :]
                        )
                        raw = npool.tile([128, 512], BF, tag=f"raw{j}", bufs=3)
                        if p == NBLK - 1:
                            nc.scalar.copy(raw[64:128, :], oX[64:128, :])
                        else:
                            nc.vector.tensor_copy(raw[64:128, :], oX[64:128, :])
                        raws.append(raw)
                    pending = (b, qb, raws, den)

            # tail: one reciprocal for the last iteration, then both norms
            rec_tail = npool.tile([33, 512], BF, tag="recb", bufs=2)
            with nc.allow_low_precision(reason="bf16 softmax 1/denom"):
                nc.vector.reciprocal(rec_tail[:], pending[3][:])
            emit_norm_head(pending, 0, rec_tail)
            emit_norm_head(pending, 1, rec_tail)

            nc.gpsimd.collective_compute(
                "AllToAll",
                ALU.bypass,
                ins=[a2a_in.opt()],
                outs=[a2a_out.opt()],
                replica_groups=[list(range(NCORES))],
            )

            # paced dummy-matmul chain: keeps the PE's activity monitor at
            # full clock across the AllToAll so proj doesn't run at 1.2GHz.
            # Three matmuls per link (~80% PE duty) paced by a small DVE
            # read-back; the light single-matmul tail links drain fast once
            # the collective lands so they don't delay proj.
            for link in range(32):
                heavy = link < 24
                dps = psum.tile([128, 1024], F32, tag="spair", bufs=3)
                nc.tensor.matmul(
                    dps[:, 0:512], garb[:, 0:128], garb[:, 0:512],
                    start=True, stop=True,
                )
                if heavy:
                    nc.tensor.matmul(
                        dps[:, 512:1024], garb[:, 0:128], garb[:, 0:512],
                        start=True, stop=True,
                    )
                    nc.tensor.matmul(
                        dps[:, 0:512], garb[:, 0:128], garb[:, 0:512],
                        start=True, stop=True,
                    )
                nc.vector.tensor_copy(gout[:, 0:64], dps[:, 0:64])

            # ---------------- proj ----------------
            rhs_sb = cpool.tile([128, EC, 512], BF)
            for kc in range(EC):
                nc.sync.dma_start(
                    rhs_sb[:, kc, :], a2a_out[kc * 128 : (kc + 1) * 128, :]
                )
            for ecn in range(EC):
                yp = psum.tile([128, 1024], F32, tag="spair", bufs=3)
                for kc in range(EC):
                    nc.tensor.matmul(
                        yp[:, 0:512],
                        wp_sb[:, kc * 1024 + ecn * 128 : kc * 1024 + (ecn + 1) * 128],
                        rhs_sb[:, kc, :],
                        start=(kc == 0),
                        stop=(kc == EC - 1),
                    )
                y_sb = ypool.tile([128, 512], F32)
                nc.vector.tensor_scalar(
                    out=y_sb[:],
                    in0=yp[:, 0:512],
                    scalar1=bias_sb[:, ecn : ecn + 1],
                    scalar2=None,
                    op0=ALU.add,
                )
                nc.sync.dma_start(out_ext[ecn * 128 : (ecn + 1) * 128, :], y_sb[:])

    _split_multi_waits(nc)
    return nc


def _make_in_maps(x, w_qkv, w_proj, b_proj):
    x = np.asarray(x, dtype=np.float32)
    w_qkv = np.asarray(w_qkv, dtype=np.float32)
    w_proj = np.asarray(w_proj, dtype=np.float32)
    b_proj = np.asarray(b_proj, dtype=np.float32)

    xT = np.ascontiguousarray(x.reshape(TOK, D).T).astype(BF16)
    wq_full = w_qkv[:, 0:D]
    wk_full = w_qkv[:, D : 2 * D]
    wv_full = w_qkv[:, 2 * D : 3 * D]

    def to_sb(wpair):  # [1024, 128] -> [128, 8*128] (e-chunk-major columns)
        return np.ascontiguousarray(
            wpair.reshape(EC, 128, 128).transpose(1, 0, 2).reshape(128, 1024)
        ).astype(BF16)

    wp_sb = np.ascontiguousarray(
        w_proj.reshape(EC, 128, 1024).transpose(1, 0, 2).reshape(128, 8192)
    ).astype(BF16)
    bias_sb = np.ascontiguousarray(b_proj.reshape(EC, 128).T).astype(np.float32)

    in_maps = []
    for c in range(NCORES):
        hA, hB = 2 * c, 2 * c + 1

        def pair(w):
            return np.concatenate(
                [w[:, hA * HD : (hA + 1) * HD], w[:, hB * HD : (hB + 1) * HD]], axis=1
            )

        in_maps.append(
            {
                "xT": xT,
                "wq": to_sb(pair(wq_full)),
                "wk": to_sb(pair(wk_full)),
                "wv": to_sb(pair(wv_full)),
                "wp": wp_sb,
                "bias": bias_sb,
                "coreid": np.array([[c]], dtype=np.uint32),
            }
        )
    return in_maps


_CACHE = {}


def kernel(x, w_qkv, w_proj, b_proj):
    import concourse.bass_utils as bass_utils

    bass_utils.upload_artifacts = lambda tmpdir: tmpdir  # no S3 in container

    if "nc" not in _CACHE:
        _CACHE["nc"] = _build_nc()
    nc = _CACHE["nc"]

    in_maps = _make_in_maps(x, w_qkv, w_proj, b_proj)

    trace = _install_axon_profile_hook()
    try:
        res = bass_utils.run_bass_kernel_spmd(
            nc, in_maps, list(range(NCORES)), trace=trace
        )
    except Exception:
        if not trace:
            raise
        res = bass_utils.run_bass_kernel_spmd(
            nc, in_maps, list(range(NCORES)), trace=False
        )

    kernel.last_exec_time_ns = res.exec_time_ns

    out = np.empty((B, N, D), dtype=np.float32)
    for c in range(NCORES):
        yT = np.asarray(res.results[c]["out"], dtype=np.float32)  # [1024, 512]
        b, s = c // 4, c % 4
        out[b, s * 512 : (s + 1) * 512, :] = yT.T
    return out


kernel.last_exec_time_ns = None


# revision 17
# speedup vs baseline: 1.1289x; 1.0125x over previous
"""Distributed Trainium2 kernel for nn_Attention (B=2, N=2048, D=1024, H=16).

Sharding: tensor-parallel over heads (2 heads per core) for qkv + attention,
then an AllToAll redistributes attention output so each core projects a
512-row slice of the output (cores 0-3: batch 0, cores 4-7: batch 1).

Structure (v6): the exp stream on ScalarE is the attention phase's hard
floor (~1.01us per [128,1024] block, 128 blocks); the Tile scheduler lays
the program out as [init | qkv(b0)+most of qkv(b1) dense | ACT-bound
attention | AllToAll | proj]. Measured lessons folded in:
  - Forcing qkv matmuls between the PV pairs slows the ACT stream ~20%
    (contention), so qkv work is emitted as coarse trailing items and the
    scheduler hoists it into the dense phase.
  - Both heads' softmax denominators land on psum partitions 0/32 (head-B
    ones column at 160), so one DVE reciprocal [33,512] covers both heads.
  - Iteration-boundary ACT bubbles come from the single-buffered PV
    accumulators: copies are ordered denA,rawA,denB,rawB so oA frees after
    two DVE ops; the final iteration's raw copies go to the idle ScalarE.
  - The ncfw warm-up AllGather is pinned late (input DMA sources
    iteration-6 data) so the real AllToAll starts hot.
  - A paced dummy-matmul chain bridges the AllToAll so proj doesn't run
    at the 1.2GHz throttled clock.
"""

import sys
import types

import numpy as np

if "/opt/trn_rl_repo" not in sys.path:
    sys.path.insert(0, "/opt/trn_rl_repo")

import ml_dtypes

B, N, D = 2, 2048, 1024
H, HD = 16, 64
SCALE = HD**-0.5
TOK = B * N  # 4096, token index = b*N + t
EC = 8  # embed-dim chunks of 128
NCORES = 8
# per k-block vones layout [128 tok, 256]:
#   [1 | 0*63 | V_A(64) | 0*32 | 1 | 0*31 | V_B(64)]
# so the PV matmul (M=128) puts head-A's softmax denominator on psum
# partition 0 and head-B's on partition 32.
VSTRIDE = 256
NKB = TOK // 128  # 32 k-blocks across both batches
NTCN = TOK // 512  # 8 qkv token tiles

BF16 = ml_dtypes.bfloat16


def _install_axon_profile_hook():
    """Best-effort: register the NTFF profile hook the RL container's antenv
    stub omits, so run_bass_kernel_spmd(trace=True) can report exec_time_ns."""
    try:
        import antenv

        if "antenv.axon_hooks" not in sys.modules:
            hooks = types.ModuleType("antenv.axon_hooks")
            hooks._hook = None
            hooks.set_axon_ntff_profile_hook = lambda h: setattr(hooks, "_hook", h)
            hooks.get_axon_ntff_profile_hook = lambda: hooks._hook
            sys.modules["antenv.axon_hooks"] = hooks
            antenv.axon_hooks = hooks
            from trn_agent_boot.trn_boot import _ntff_profile_via_ctypes

            hooks.set_axon_ntff_profile_hook(
                _ntff_profile_via_ctypes("/opt/axon/libaxon_pjrt.so")
            )
        return True
    except Exception:
        return False


def _split_multi_waits(nc):
    """neuronxcc's walrus (CoreV3 setupSyncWait) rejects instructions that
    carry more than one semaphore wait, but Tile's wait assignment freely
    attaches several. Hoist the extra waits onto freshly inserted same-engine
    NoOps placed directly before the instruction — the engine stalls at the
    same program point, so semantics are unchanged."""
    import concourse.mybir as mybir

    n_split = 0
    for fn in nc.m.functions:
        for bb in fn.blocks:
            insts = bb.instructions
            if not any(
                i.sync_info is not None and len(i.sync_info.on_wait) > 1
                for i in insts
            ):
                continue
            new_insts = []
            for ins in insts:
                si = ins.sync_info
                if si is not None and len(si.on_wait) > 1:
                    waits = list(si.on_wait)
                    for w in waits[:-1]:
                        nop = mybir.InstNoOp(
                            name=f"wsplit-{n_split}",
                            engine=ins.engine,
                            ins=[],
                            outs=[],
                            sync_info=mybir.SyncInfo(on_wait=[w], on_update=[]),
                        )
                        new_insts.append(nop)
                        n_split += 1
                    ins.sync_info = mybir.SyncInfo(
                        on_wait=[waits[-1]], on_update=list(si.on_update)
                    )
                new_insts.append(ins)
            bb.instructions = new_insts


def _build_nc():
    import concourse.bass as bass
    import concourse.mybir as mybir
    import concourse.tile as tile

    F32 = mybir.dt.float32
    BF = mybir.dt.bfloat16
    AF = mybir.ActivationFunctionType
    ALU = mybir.AluOpType

    nc = bass.Bass()
    xT_ext = nc.declare_dram_parameter("xT", [D, TOK], BF, isOutput=False)
    wq_ext = nc.declare_dram_parameter("wq", [128, 1024], BF, isOutput=False)
    wk_ext = nc.declare_dram_parameter("wk", [128, 1024], BF, isOutput=False)
    wv_ext = nc.declare_dram_parameter("wv", [128, 1024], BF, isOutput=False)
    wp_ext = nc.declare_dram_parameter("wp", [128, 8192], BF, isOutput=False)
    bias_ext = nc.declare_dram_parameter("bias", [128, 8], F32, isOutput=False)
    coreid_ext = nc.declare_dram_parameter(
        "coreid", [1, 1], mybir.dt.uint32, isOutput=False
    )
    out_ext = nc.declare_dram_parameter("out", [D, 512], F32, isOutput=True)

    with tile.TileContext(nc) as tc:
        with (
            tc.tile_pool(name="const", bufs=1) as cpool,
            tc.tile_pool(name="e", bufs=6) as epool,
            tc.tile_pool(name="norm", bufs=2) as npool,
            tc.tile_pool(name="y", bufs=2) as ypool,
            tc.tile_pool(name="psum", bufs=2, space="PSUM") as psum,
            tc.tile_pool(name="dram", bufs=1, space="DRAM") as dram,
        ):
            wq_sb = cpool.tile([128, 1024], BF)
            wk_sb = cpool.tile([128, 1024], BF)
            wv_sb = cpool.tile([128, 1024], BF)
            wp_sb = cpool.tile([128, 8192], BF)
            bias_sb = cpool.tile([128, 8], F32)
            qt_sb = cpool.tile([128, TOK], BF)
            kt_sb = cpool.tile([128, TOK], BF)
            vones = cpool.tile([128, NKB, VSTRIDE], BF)
            x_sb = [
                cpool.tile([128, EC, 512], BF, name=f"x{t}") for t in range(NTCN)
            ]
            garb = cpool.tile([128, 512], BF)
            gout = cpool.tile([128, 512], BF)

            nc.vector.memset(garb[:], 0.0)
            nc.vector.memset(vones[:], 0.0)
            nc.vector.memset(vones[:, :, 0:1], 1.0)
            nc.vector.memset(vones[:, :, 160:161], 1.0)
            bones = cpool.tile([33, 64], BF)
            nc.vector.memset(bones[0:1, :], 1.0)
            nc.vector.memset(bones[32:33, :], 1.0)

            # ---------------- input DMA issue schedule ----------------
            # sync gets the weights + even x(b0) chunks; scalar (idle until
            # the first exp) gets the odd chunks. x(b1) is issued from
            # inside attention iterations 0-1 on sync; wp/bias mid-attention
            # (the prologue window is HBM-bandwidth limited: 8 cores pull
            # replicated x concurrently).
            nc.sync.dma_start(wq_sb[:, 0:512], wq_ext[:, 0:512])
            nc.sync.dma_start(wq_sb[:, 512:1024], wq_ext[:, 512:1024])
            for ec in range(0, EC, 2):
                nc.sync.dma_start(
                    x_sb[0][:, ec, :], xT_ext[ec * 128 : (ec + 1) * 128, 0:512]
                )
            nc.sync.dma_start(wk_sb[:], wk_ext[:])
            nc.sync.dma_start(wv_sb[:], wv_ext[:])
            for tcn in range(1, 4):
                for ec in range(0, EC, 2):
                    nc.sync.dma_start(
                        x_sb[tcn][:, ec, :],
                        xT_ext[ec * 128 : (ec + 1) * 128, tcn * 512 : (tcn + 1) * 512],
                    )
            for tcn in range(4):
                for ec in range(1, EC, 2):
                    nc.scalar.dma_start(
                        x_sb[tcn][:, ec, :],
                        xT_ext[ec * 128 : (ec + 1) * 128, tcn * 512 : (tcn + 1) * 512],
                    )

            # ---------------- qkv emission helpers ----------------
            def emit_QK(t, wsb, dst):
                ps = psum.tile([128, 1024], F32, tag="spair", bufs=3)
                for ec in range(EC):
                    nc.tensor.matmul(
                        ps[:, 0:512],
                        wsb[:, ec * 128 : (ec + 1) * 128],
                        x_sb[t][:, ec, :],
                        start=(ec == 0),
                        stop=(ec == EC - 1),
                    )
                nc.vector.tensor_copy(dst[:, t * 512 : (t + 1) * 512], ps[:, 0:512])

            def emit_Q(t):
                emit_QK(t, wq_sb, qt_sb)

            def emit_K(t):
                emit_QK(t, wk_sb, kt_sb)

            def emit_V(t):
                ps = psum.tile([128, 1024], F32, tag="spair", bufs=3)
                for tsub in range(4):
                    for ec in range(EC):
                        nc.tensor.matmul(
                            ps[:, tsub * 128 : tsub * 128 + 128],
                            x_sb[t][:, ec, tsub * 128 : (tsub + 1) * 128],
                            wv_sb[:, ec * 128 : (ec + 1) * 128],
                            start=(ec == 0),
                            stop=(ec == EC - 1),
                        )
                for tsub in range(4):
                    g = t * 4 + tsub
                    nc.vector.tensor_copy(
                        vones[:, g, 64:128], ps[:, tsub * 128 : tsub * 128 + 64]
                    )
                    nc.vector.tensor_copy(
                        vones[:, g, 192:256], ps[:, tsub * 128 + 64 : tsub * 128 + 128]
                    )

            # ---------------- prologue: minimal batch-0 qkv ----------------
            emit_Q(0)
            emit_K(0)
            emit_V(0)
            emit_K(1)
            emit_V(1)

            # feeder schedule: (iter, kb) -> list of qkv emitters, placed as
            # coarse trailing items; the Tile scheduler hoists them into the
            # dense region where the PE has slack.
            FEED = {
                (0, 0): [lambda: emit_K(2)],
                (0, 3): [lambda: emit_V(2)],
                (0, 6): [lambda: emit_K(3)],
                (0, 9): [lambda: emit_V(3)],
                (0, 13): [lambda: emit_Q(1)],
                (1, 0): [lambda: emit_Q(2)],
                (1, 4): [lambda: emit_K(4)],
                (1, 8): [lambda: emit_V(4)],
                (1, 12): [lambda: emit_Q(3)],
                (2, 0): [lambda: emit_K(5)],
                (2, 4): [lambda: emit_V(5)],
                (2, 8): [lambda: emit_K(6)],
                (3, 0): [lambda: emit_K(7)],
                (3, 4): [lambda: emit_V(6)],
                (3, 8): [lambda: emit_Q(4)],
                (4, 0): [lambda: emit_V(7)],
                (4, 8): [lambda: emit_Q(5)],
                (5, 6): [lambda: emit_Q(6)],
                (6, 6): [lambda: emit_Q(7)],
            }
            # x(b1) chunk DMAs issued on sync from iters 0-1
            XB1 = {}
            for i, t in enumerate(range(4, 8)):
                for ec in range(EC):
                    slot = i * EC + ec  # 0..31 over iters 0-1
                    XB1.setdefault((slot // 16, slot % 16), []).append((t, ec))

            # ---------------- attention ----------------
            a2a_in = dram.tile([1024, 512], BF)
            a2a_out = dram.tile([1024, 512], BF)
            warm_in = dram.tile([1, 512], BF)
            warm_out = dram.tile([8, 512], BF)

            def emit_norm_head(pend, j, rec_in):
                """Normalize one head of a finished (b, qb) iteration's raw
                attention output; overlapped with the next iteration."""
                pb, pqb, raws, _den = pend
                p0 = 32 * j
                bcp = psum.tile([128, 512], F32, tag="spair", bufs=3)
                nc.tensor.matmul(
                    bcp[64:128, :],
                    bones[p0 : p0 + 1, 0:64],
                    rec_in[p0 : p0 + 1, :],
                    start=True,
                    stop=True,
                )
                onorm = npool.tile([128, 512], BF, tag="onorm")
                nc.vector.tensor_mul(
                    onorm[64:128, :], raws[j][64:128, :], bcp[64:128, :]
                )
                row = 128 * (4 * pb + pqb) + 64 * j
                nc.sync.dma_start(a2a_in[row : row + 64, :], onorm[64:128, :])

            def emit_scores(b, qb, kb):
                qoff = b * N + qb * 512
                koff = b * N + kb * 128
                sp = psum.tile([128, 1024], F32, tag="spair", bufs=3)
                nc.tensor.matmul(
                    sp[:, 0:512],
                    kt_sb[0:64, koff : koff + 128],
                    qt_sb[0:64, qoff : qoff + 512],
                    start=True,
                    stop=True,
                )
                nc.tensor.matmul(
                    sp[:, 512:1024],
                    kt_sb[64:128, koff : koff + 128],
                    qt_sb[64:128, qoff : qoff + 512],
                    start=True,
                    stop=True,
                )
                e_t = epool.tile([128, 1024], BF)
                nc.scalar.activation(e_t[:], sp[:], AF.Exp, scale=SCALE)
                return e_t

            iters = [(b, qb) for b in range(B) for qb in range(N // 512)]
            pending = None
            e_carry = None
            rec_cur = None
            warm_src = None
            for it_idx, (b, qb) in enumerate(iters):
                oA = psum.tile([128, 512], F32, tag="oA", bufs=1)
                oB = psum.tile([128, 512], F32, tag="oB", bufs=1)
                for kb in range(N // 128):
                    g = b * (N // 128) + kb
                    if kb == 0:
                        if e_carry is not None:
                            e_t = e_carry
                            e_carry = None
                        else:
                            e_t = emit_scores(b, qb, 0)
                    last = kb == (N // 128) - 1
                    # one-kb software pipelining: the NEXT block's scores+exp
                    # are emitted ahead of this block's PV pair
                    if not last:
                        e_next = emit_scores(b, qb, kb + 1)
                    elif it_idx + 1 < len(iters):
                        e_carry = emit_scores(*iters[it_idx + 1], 0)
                        e_next = None
                    else:
                        e_next = None
                    nc.tensor.matmul(
                        oA[:],
                        vones[:, g, 0:128],
                        e_t[:, 0:512],
                        start=(kb == 0),
                        stop=last,
                    )
                    nc.tensor.matmul(
                        oB[:],
                        vones[:, g, 128:256],
                        e_t[:, 512:1024],
                        start=(kb == 0),
                        stop=last,
                    )
                    if e_next is not None:
                        e_t = e_next
                    for (t, ec) in XB1.get((it_idx, kb), ()):
                        nc.sync.dma_start(
                            x_sb[t][:, ec, :],
                            xT_ext[
                                ec * 128 : (ec + 1) * 128, t * 512 : (t + 1) * 512
                            ],
                        )
                    for fn in FEED.get((it_idx, kb), ()):
                        fn()
                    if (it_idx, kb) == (5, 0):
                        # proj weights: issued mid-attention when HBM is idle
                        nc.sync.dma_start(wp_sb[:], wp_ext[:])
                        nc.sync.dma_start(bias_sb[:], bias_ext[:])
                    if kb == 3 and pending is not None:
                        # one reciprocal covers both heads (dens on
                        # partitions 0 and 32)
                        rec_cur = npool.tile([33, 512], BF, tag="recb", bufs=2)
                        with nc.allow_low_precision(reason="bf16 softmax 1/denom"):
                            nc.vector.reciprocal(rec_cur[:], pending[3][:])
                    if kb == 8 and pending is not None:
                        emit_norm_head(pending, 0, rec_cur)
                    if kb == 10 and it_idx == 6 and pending is not None:
                        # keep a late-written tile as the warm-collective DMA
                        # source so the scheduler cannot hoist the ncfw
                        # warm-up earlier than ~iteration 6
                        warm_src = pending[2][0]
                    if kb == 12 and pending is not None:
                        emit_norm_head(pending, 1, rec_cur)
                        pending = None
                    if kb == 13 and it_idx == 6 and warm_src is not None:
                        nc.sync.dma_start(warm_in[:], warm_src[64:65, 0:512])
                        nc.gpsimd.collective_compute(
                            "AllGather",
                            ALU.bypass,
                            ins=[warm_in.opt()],
                            outs=[warm_out.opt()],
                            replica_groups=[list(range(NCORES))],
                        )
                # stash raw output + denominators in SBUF so the psum
                # accumulators free; ordered oA-first (den A then raw A) so
                # the next iteration's PV restarts on oA after two DVE ops.
                # On the final iteration the raw copies run on the now-idle
                # Scalar engine so the DVE starts the reciprocal immediately.
                den = npool.tile([33, 512], F32, tag="den", bufs=3)
                raws = []
                final = it_idx == len(iters) - 1
                for j, oX in ((0, oA), (1, oB)):
                    nc.vector.tensor_copy(
                        den[32 * j : 32 * j + 1, :], oX[32 * j : 32 * j + 1, :]
                    )
                    raw = npool.tile([128, 512], BF, tag=f"raw{j}", bufs=3)
                    if final:
                        nc.scalar.copy(raw[64:128, :], oX[64:128, :])
                    else:
                        nc.vector.tensor_copy(raw[64:128, :], oX[64:128, :])
                    raws.append(raw)
                pending = (b, qb, raws, den)

            # tail: one reciprocal for the last iteration, then both norms
            rec_tail = npool.tile([33, 512], BF, tag="recb", bufs=2)
            with nc.allow_low_precision(reason="bf16 softmax 1/denom"):
                nc.vector.reciprocal(rec_tail[:], pending[3][:])
            emit_norm_head(pending, 0, rec_tail)
            emit_norm_head(pending, 1, rec_tail)

            nc.gpsimd.collective_compute(
                "AllToAll",
                ALU.bypass,
                ins=[a2a_in.opt()],
                outs=[a2a_out.opt()],
                replica_groups=[list(range(NCORES))],
            )

            # paced dummy-matmul chain: keeps the PE's activity monitor at
            # full clock across the AllToAll so proj doesn't run at 1.2GHz
            for _ in range(20):
                dps = psum.tile([128, 1024], F32, tag="spair", bufs=3)
                nc.tensor.matmul(
                    dps[:, 0:512], garb[:, 0:128], garb[:, 0:512],
                    start=True, stop=True,
                )
                nc.vector.tensor_copy(gout[:], dps[:, 0:512])

            # ---------------- proj ----------------
            rhs_sb = cpool.tile([128, EC, 512], BF)
            for kc in range(EC):
                nc.sync.dma_start(
                    rhs_sb[:, kc, :], a2a_out[kc * 128 : (kc + 1) * 128, :]
                )
            for ecn in range(EC):
                yp = psum.tile([128, 1024], F32, tag="spair", bufs=3)
                for kc in range(EC):
                    nc.tensor.matmul(
                        yp[:, 0:512],
                        wp_sb[:, kc * 1024 + ecn * 128 : kc * 1024 + (ecn + 1) * 128],
                        rhs_sb[:, kc, :],
                        start=(kc == 0),
                        stop=(kc == EC - 1),
                    )
                y_sb = ypool.tile([128, 512], F32)
                nc.vector.tensor_scalar(
                    out=y_sb[:],
                    in0=yp[:, 0:512],
                    scalar1=bias_sb[:, ecn : ecn + 1],
                    scalar2=None,
                    op0=ALU.add,
                )
                nc.sync.dma_start(out_ext[ecn * 128 : (ecn + 1) * 128, :], y_sb[:])

    _split_multi_waits(nc)
    return nc


def _make_in_maps(x, w_qkv, w_proj, b_proj):
    x = np.asarray(x, dtype=np.float32)
    w_qkv = np.asarray(w_qkv, dtype=np.float32)
    w_proj = np.asarray(w_proj, dtype=np.float32)
    b_proj = np.asarray(b_proj, dtype=np.float32)

    xT = np.ascontiguousarray(x.reshape(TOK, D).T).astype(BF16)
    wq_full = w_qkv[:, 0:D]
    wk_full = w_qkv[:, D : 2 * D]
    wv_full = w_qkv[:, 2 * D : 3 * D]

    def to_sb(wpair):  # [1024, 128] -> [128, 8*128] (e-chunk-major columns)
        return np.ascontiguousarray(
            wpair.reshape(EC, 128, 128).transpose(1, 0, 2).reshape(128, 1024)
        ).astype(BF16)

    wp_sb = np.ascontiguousarray(
        w_proj.reshape(EC, 128, 1024).transpose(1, 0, 2).reshape(128, 8192)
    ).astype(BF16)
    bias_sb = np.ascontiguousarray(b_proj.reshape(EC, 128).T).astype(np.float32)

    in_maps = []
    for c in range(NCORES):
        hA, hB = 2 * c, 2 * c + 1

        def pair(w):
            return np.concatenate(
                [w[:, hA * HD : (hA + 1) * HD], w[:, hB * HD : (hB + 1) * HD]], axis=1
            )

        in_maps.append(
            {
                "xT": xT,
                "wq": to_sb(pair(wq_full)),
                "wk": to_sb(pair(wk_full)),
                "wv": to_sb(pair(wv_full)),
                "wp": wp_sb,
                "bias": bias_sb,
                "coreid": np.array([[c]], dtype=np.uint32),
            }
        )
    return in_maps


_CACHE = {}


def kernel(x, w_qkv, w_proj, b_proj):
    import concourse.bass_utils as bass_utils

    bass_utils.upload_artifacts = lambda tmpdir: tmpdir  # no S3 in container

    if "nc" not in _CACHE:
        _CACHE["nc"] = _build_nc()
    nc = _CACHE["nc"]

    in_maps = _make_in_maps(x, w_qkv, w_proj, b_proj)

    trace = _install_axon_profile_hook()
    try:
        res = bass_utils.run_bass_kernel_spmd(
            nc, in_maps, list(range(NCORES)), trace=trace
        )
    except Exception:
        if not trace:
            raise
        res = bass_utils.run_bass_kernel_spmd(
            nc, in_maps, list(range(NCORES)), trace=False
        )

    kernel.last_exec_time_ns = res.exec_time_ns

    out = np.empty((B, N, D), dtype=np.float32)
    for c in range(NCORES):
        yT = np.asarray(res.results[c]["out"], dtype=np.float32)  # [1024, 512]
        b, s = c // 4, c % 4
        out[b, s * 512 : (s + 1) * 512, :] = yT.T
    return out


kernel.last_exec_time_ns = None
